# revision 57
# baseline (speedup 1.0000x reference)
"""Dual (global + local-masked) BERT self-attention on 8 Trainium2 NeuronCores.

Problem: B=2, S=2048, H=1024, NH=16 heads of DH=64.
  q/k/v = hidden @ W{q,k,v}.T + b ; scores = q k^T / 8
  probs_g = softmax(scores + attention_mask)         (additive, zeros in spec)
  probs_l = softmax(scores + (-inf where local_mask==0))
  out     = gate * (probs_l @ v) + (1-gate) * (probs_g @ v)

Sharding: 32 (batch, head) pairs -> 4 heads per core (core c: batch c//4,
heads 4*(c%4)..+4). Each core computes its heads' projections + dual
attention independently; no collectives.

Per-core kernel: v4 (_build4) is the default path for the spec inputs
(zero attention_mask, zero biases); the v3 path below remains as the
fallback for use_em/has_b variants. v4 on top of v3:
  - fp8e4m3 DoubleRow projections with residual compensation (see the
    _build4 docstring): proj PE cost 41us -> 30.7us, K=256 per matmul.
  - 2-engine parallel input feed (SP + Pool HW queues overlap in the DMA
    engines): w+x8+mask-q1 on SP, x8s0+rx8+first mask quarters on Pool.
  - Minimal prefix (pair-0 Q/K first 1024 cols only) + chunk order
    (0,0),(1,0),(0,1),(1,1),(2,0),(3,0),(2,1),(3,1); V + K0nq1 fill
    chunk 0 (dependency-forced), remaining projections spread over
    chunks 1-5 as 1-term pieces per iteration.
  - A dummy 1-col matmul at t~0.3us starts the PE p-state ramp clock so
    the first projections run at full clock (the model ramps over 3us).
  - bf16 mask (el muls hit DVE 2x_1p: 594 -> 327ns; Pool keeps hf1).
  - Final-chunk epilogue: staging split DVE/ACT, 4 small out-DMAs
    alternating SP/ACT.
Engine budget (CoreSim): PE 142.2us busy (scores 54.6 + ctx 55.4 + proj
30.7), ACT 135.9 (128 exps @ 1038ns = floor), DVE 78, Pool 70. Span
157.8us: lead 3.6 + PE-bound prefix+chunk0 ~32.5 + 7 ACT-floor chunks
116.2 + tail 5.0. rel err 2.7e-3 measured on HW. Structural walls: the
ACT exp floor (1024-col tiles; wider tiles or deeper score lookahead
need >8 PSUM banks -- accs 3 + scores 4 + aux 1 are exactly 8, and
start=True zeroes whole banks so regions can't share), and chunk-0's
forced PE load (all 16 V fills + K0nq1 must land inside it).

Per-core kernel (v3, the fallback path):
  - DMAs issued from the (otherwise idle) Pool engine, in consumption
    order: xt/wq/wk tiles interleaved (projections stream tile t as it
    lands), then wv, then small tensors, then the mask LAST. The mask
    travels as fp8 (0/1 is exact) to halve its 8MB -> 4MB of HBM traffic.
  - Projections Q^T/K^T per head-pair with 4 PSUM chunks accumulating
    t-OUTER so PE consumption matches the DMA feed; proj of pair 1 is
    deferred until after the pair-0 attention so ACT's exp stream starts
    ~15us earlier. Zero-bias projections copy PSUM->SBUF on DVE, keeping
    ACT = pure exp.
  - Attention per (head, 1024-q chunk): scores^T [128 keys, 1024 q] on PE
    (K=64), software-pipelined 2 tiles ahead (psc bufs=2); ONE 1024-wide
    exp on ACT shared by both branches; e_l = e * mask split DVE/Pool.
  - ctx FLIPPED: out[q, d] = sum_k e[k,q] V[k,d] via lhsT = 128-q slabs of
    e, rhs = [V_tile | ones] (65 cols -> denominators land per-partition).
    16 accumulators packed 6/6/4 into THREE PSUM banks (only the bank's
    first group sets start=True: start zeroes the whole 2KB bank), which
    frees one spare bank: the V projection streams through it interleaved
    into the FIRST attention chunk, and the pair-1 Q/K projection streams
    through it interleaved into the h=1/h=2 chunks — ACT's exp stream runs
    gap-free from ~26us to the end (attention is exp-bound: ACT is the
    single engine that can do exp, 853ns per [128,1024] tile).
  - All prologue DMAs are issued from SP: the issue stream is throttled by
    DMA-ring slots, so issuing from a compute engine would block that
    engine's instruction queue for ~40us.
  - Epilogue (emitted deferred, inside the NEXT chunk's loop so its ops
    queue behind that chunk's first el muls): tiny [128,4] reciprocals of
    per-partition denominators, gate coefs on Pool, scale+combine via
    tensor_scalar/scalar_tensor_tensor with per-partition scalars, one
    [128, 8, 64] staging tile and ONE out-DMA per chunk. No broadcasts,
    no selector matmuls, no row-extraction DMAs.
No max-subtraction in softmax: scores are O(+-5), exp is safe in fp32 and
softmax is shift-invariant, so the reference is matched to ~4e-3.
The scores pipeline is continuous across chunk boundaries (the emitter
needs only qt/kt slices, so the next chunk's first scores are issued
during the current chunk's last iterations). Input transfers go as
2-tile pairs: the DMA issue stream is paced by ring-slot waits at the
transfer rate, so halving the transfer count halves the per-transfer
fixed latencies in the feed-bound lead (4-tile groups are worse: the
t-outer projection loses streaming granularity).
Output per core: [2048 q, 256 dims] f32 (natural layout); host reassembles.
The final chunk's epilogue is flushed inside the PSUM-pool scope with its
staging copies split DVE/ACT to shorten the tail chain.
Each chunk's output DMA is split in halves so the first transfer overlaps
the second half's scale ops.
CoreSim cost model: v4 157.8us/core (v3 176.7, first layout 392); measured
rel err 2.7e-3 (v4) / 3.5e-3 (v3).
"""

import sys

sys.path.insert(0, "/opt/trn_rl_repo")

import numpy as np
import ml_dtypes

B, S, H, NH, DH = 2, 2048, 1024, 16, 64
NCORES = 8
HPC = 4          # heads per core
MPC = HPC // 2   # head pairs per core
QC = 1024        # query chunk (free dim of scores/ctx psums)
NQC = S // QC
KT = S // 128    # key tiles
XT_T = H // 128  # X^T k-tiles for projections

_BUILT = {}


def _build(use_em: bool, repeat: int = 1, pairs: bool = False, abl: str = "", qcw: int = QC, ctxbufs: int = 1, scbufs: int = 2, tune: bool = True, has_b: bool = False, gbc: bool = False, v2: bool = True, la: int = 3, hw2: int = 512, v3: bool = True):
    if v3:
        v2 = True
    from contextlib import ExitStack

    import concourse.mybir as mybir
    from concourse import bacc, tile

    f32 = mybir.dt.float32
    bf16 = mybir.dt.bfloat16
    AF = mybir.ActivationFunctionType

    nc = bacc.Bacc("TRN2", target_bir_lowering=False, debug=False)

    xt_d = nc.dram_tensor("xt", [H, S], bf16, kind="ExternalInput").ap()
    wq_d = nc.dram_tensor("wq", [H, 256], bf16, kind="ExternalInput").ap()
    wk_d = nc.dram_tensor("wk", [H, 256], bf16, kind="ExternalInput").ap()
    wv_d = nc.dram_tensor("wv", [H, 256], bf16, kind="ExternalInput").ap()
    bqk_d = nc.dram_tensor("bqk", [2, 256], f32, kind="ExternalInput").ap()
    bv_d = nc.dram_tensor("bv", [1, 256], bf16, kind="ExternalInput").ap()
    f8 = mybir.dt.float8e4
    msk_d = nc.dram_tensor("msk", [KT, 128, S], bf16, kind="ExternalInput").ap()
    msk8_d = nc.dram_tensor("msk8", [KT, 128, S], f8, kind="ExternalInput").ap()
    # gt[r, h, q]: head h, r = (gate_h, 1-gate_h)
    gt_d = nc.dram_tensor("gt", [2, HPC, S], f32, kind="ExternalInput").ap()
    # gt2: gate rows only, single-partition bf16 layout for the v2 epilogue
    gt2_d = nc.dram_tensor("gt2", [1, HPC, S], bf16, kind="ExternalInput").ap()
    # sel[r, j, d] = 1.0 if r == j else 0 — K=2 broadcast selectors
    sel_d = nc.dram_tensor("sel", [4, 4, 64], bf16, kind="ExternalInput").ap()
    if use_em:
        em_d = nc.dram_tensor("em", [KT, 128], f32, kind="ExternalInput").ap()
    # gp[p, h, sub]: gate for q = sub*128 + p, partition-major for v3
    gp_d = nc.dram_tensor("gp", [128, HPC, S // 128], f32, kind="ExternalInput").ap()
    if v3:
        out_d = nc.dram_tensor("out", [S, HPC * DH], f32, kind="ExternalOutput").ap()
    else:
        out_d = nc.dram_tensor("out", [HPC * DH, S], f32, kind="ExternalOutput").ap()

    with tile.TileContext(nc) as tc, ExitStack() as ctx:
        big = ctx.enter_context(tc.tile_pool(name="big", bufs=1))

        # SP issues all prologue DMAs: the issue stream is throttled by the
        # DMA queues (each issue waits for ring slots), so putting it on a
        # compute engine would block that engine's queue for ~40us.
        dma_issue = nc.sync.dma_start

        xt_sb = big.tile([128, XT_T, S], bf16, name="xt_sb")
        w_sbs = {}
        for nm in ("wq", "wk", "wv"):
            w_sbs[nm] = big.tile([128, XT_T, 256], bf16, name=f"{nm}_sb")
        w_ds = {"wq": wq_d, "wk": wk_d, "wv": wv_d}
        # interleaved in consumption order. xt travels as column halves:
        # the pre-attention projection only needs the first 1024 columns
        # (nq0); the second half feeds the in-attention nq1 fills.
        if v3:
            # paired tiles per transfer: halves the ring-slot count and the
            # per-transfer fixed latencies that pace the issue stream, while
            # keeping 2-tile streaming granularity for the t-outer projection
            for t in range(0, XT_T, 2):
                dma_issue(xt_sb[:, t:t + 2, :],
                          xt_d[t * 128:(t + 2) * 128, :].rearrange(
                              "(t p) s -> p t s", p=128))
                for nm in ("wq", "wk"):
                    dma_issue(w_sbs[nm][:, t:t + 2, :],
                              w_ds[nm][t * 128:(t + 2) * 128, :].rearrange(
                                  "(t p) s -> p t s", p=128))
            for t in range(0, XT_T, 2):
                dma_issue(w_sbs["wv"][:, t:t + 2, :],
                          w_ds["wv"][t * 128:(t + 2) * 128, :].rearrange(
                              "(t p) s -> p t s", p=128))
        else:
            for t in range(XT_T):
                dma_issue(xt_sb[:, t, :], xt_d[t * 128:(t + 1) * 128, :])
                for nm in ("wq", "wk"):
                    dma_issue(w_sbs[nm][:, t, :], w_ds[nm][t * 128:(t + 1) * 128, :])
            for t in range(XT_T):
                dma_issue(w_sbs["wv"][:, t, :], w_ds["wv"][t * 128:(t + 1) * 128, :])
        # mask right after wv: with V and pair-1 proj interleaved into the
        # attention, the first el muls need msk tiles from ~20us on.
        if v2:
            msk_sb = big.tile([128, KT, S], f8, name="msk_sb")
            if v3:
                for t in range(0, KT, 2):
                    dma_issue(msk_sb[:, t:t + 2, :],
                              msk8_d[t:t + 2].rearrange("t p s -> p t s"))
            else:
                for t in range(KT):
                    dma_issue(msk_sb[:, t, :], msk8_d[t])
        else:
            msk_sb = big.tile([128, KT, S], bf16, name="msk_sb")
            for t in range(KT):
                dma_issue(msk_sb[:, t, :], msk_d[t])
        sel_sb = big.tile([4, 4, 64], bf16, name="sel_sb")
        dma_issue(sel_sb, sel_d)
        bqk_sb = big.tile([128, 2, 2], f32, name="bqk_sb")
        dma_issue(
            bqk_sb, bqk_d.rearrange("c (t p) -> p c t", p=128)
        )
        bv_sb = big.tile([1, 256], bf16, name="bv_sb")
        dma_issue(bv_sb, bv_d)
        if use_em:
            em_sb = big.tile([128, KT], f32, name="em_sb")
            dma_issue(em_sb, em_d.rearrange("t p -> p t"))
        if v2:
            if v3:
                gp_sb = big.tile([128, HPC, S // 128], f32, name="gp_sb")
                dma_issue(gp_sb, gp_d)
            else:
                gt2_sb = big.tile([1, HPC, S], bf16, name="gt2_sb")
                dma_issue(gt2_sb, gt2_d)

        ones_r = big.tile([1, 128], bf16, name="ones_r")
        nc.vector.memset(ones_r, 1.0)

        qt_sb = big.tile([128, MPC, S], bf16, name="qt_sb")
        kt_sb = big.tile([128, MPC, S], bf16, name="kt_sb")
        v_sb = big.tile([128, KT, HPC, 65], bf16, name="v_sb")
        nc.vector.memset(v_sb[:, :, :, 64:65], 1.0)

        for _rep in range(repeat):
            # ---- projections: Q^T, K^T (transposed), V (natural) ----
            if v3:
                # v3: flipped-ctx attention. Projections per head-pair are
                # interleaved with attention so ACT's exp stream starts early.
                def proj_qk3(m, nqs=(0, 1)):
                    # PSUM chunks accumulate t-outer so PE consumption
                    # matches the DMA feed rate.
                    with tc.tile_pool(name=f"pproj{m}", bufs=1, space="PSUM") as pproj:
                        chunks = []
                        for nq in nqs:
                            for ci, (wn, dst) in enumerate(
                                    (("wq", qt_sb), ("wk", kt_sb))):
                                ps = pproj.tile([128, 1024], f32, tag=f"pp{nq}{ci}")
                                chunks.append((nq, ci, wn, dst, ps))
                        for t in range(XT_T):
                            for nq, ci, wn, dst, ps in chunks:
                                w_sb = w_sbs[wn]
                                for hlf in range(2):
                                    nc.tensor.matmul(
                                        ps[:, hlf * 512:(hlf + 1) * 512],
                                        lhsT=w_sb[:, t, m * 128:(m + 1) * 128],
                                        rhs=xt_sb[:, t, nq * 1024 + hlf * 512:
                                                  nq * 1024 + (hlf + 1) * 512],
                                        start=(t == 0), stop=(t == XT_T - 1))
                        for i, (nq, ci, wn, dst, ps) in enumerate(chunks):
                            if has_b:
                                nc.scalar.activation(
                                    dst[:, m, nq * 1024:(nq + 1) * 1024], ps,
                                    AF.Identity, bias=bqk_sb[:, ci, m:m + 1],
                                    scale=1.0)
                            elif i % 2 == 0:
                                # zero bias: qt copies on DVE, kt on ACT — the
                                # two copies gating the first scores then run
                                # in parallel (ACT's exp stream starts later).
                                nc.vector.tensor_copy(
                                    dst[:, m, nq * 1024:(nq + 1) * 1024], ps)
                            else:
                                nc.scalar.activation(
                                    dst[:, m, nq * 1024:(nq + 1) * 1024], ps,
                                    AF.Copy)

                def proj_v3():
                    with tc.tile_pool(name="pv", bufs=2, space="PSUM") as pv:
                        for st in range(KT):
                            ps = pv.tile([128, 256], f32, tag="pv")
                            for t in range(XT_T):
                                nc.tensor.matmul(
                                    ps,
                                    lhsT=xt_sb[:, t, st * 128:(st + 1) * 128],
                                    rhs=w_sbs["wv"][:, t, :],
                                    start=(t == 0),
                                    stop=(t == XT_T - 1 and not has_b))
                            if has_b:
                                nc.tensor.matmul(
                                    ps, lhsT=ones_r, rhs=bv_sb, start=False,
                                    stop=True)
                            nc.vector.tensor_copy(
                                v_sb[:, st, :, 0:64],
                                ps.rearrange("p (h d) -> p h d", h=HPC))

                rep3 = ExitStack()
                pe = rep3.enter_context(tc.tile_pool(name="pe", bufs=6))
                pel = rep3.enter_context(tc.tile_pool(name="pel", bufs=8))
                pst = rep3.enter_context(tc.tile_pool(name="pst", bufs=2))
                pc2 = rep3.enter_context(tc.tile_pool(name="pc2", bufs=2))
                po2 = rep3.enter_context(tc.tile_pool(name="po2", bufs=2))
                state = {"pending": None}

                # 16 ctx accumulators (a = br*8 + j; br 0=g 1=l) packed into
                # THREE PSUM banks (6/6/4 slots) to free one bank for the
                # interleaved pair-1 projection fills.
                ACC_SIZES = (6, 6, 4)

                def acc_loc(a):
                    if a < 6:
                        return 0, a
                    if a < 12:
                        return 1, a - 6
                    return 2, a - 12

                ACC_FIRST = {0, 6, 12}
                # coef runs: (tile, s0, s1, branch, j0)
                COEF_RUNS = ((0, 0, 6, "g", 0), (1, 0, 2, "g", 6),
                             (1, 2, 6, "l", 0), (2, 0, 4, "l", 4))

                def make_epi3(stg, h, qc):
                    def emit():
                        q0 = qc * QC
                        recs, cf = {}, {}
                        for ti, sz in enumerate(ACC_SIZES):
                            r = pc2.tile([128, sz], f32, name=f"rec{ti}",
                                         tag=f"rec{ti}")
                            nc.vector.reciprocal_approx_fast(r, stg[ti][:, :, 64])
                            recs[ti] = r
                            cf[ti] = pc2.tile([128, sz], f32, name=f"cf{ti}",
                                              tag=f"cf{ti}")
                        for ti, s0, s1, br, j0 in COEF_RUNS:
                            gs = gp_sb[:, h, qc * 8 + j0:qc * 8 + j0 + (s1 - s0)]
                            if br == "l":
                                # coef_l = g * rec_l
                                nc.gpsimd.tensor_mul(
                                    cf[ti][:, s0:s1], recs[ti][:, s0:s1], gs)
                            else:
                                # coef_g = rec_g - g * rec_g
                                cgt = pc2.tile([128, s1 - s0], f32, name="cgt",
                                               tag=f"cgt{ti}{s0}")
                                nc.gpsimd.tensor_mul(cgt, recs[ti][:, s0:s1], gs)
                                nc.gpsimd.tensor_sub(
                                    cf[ti][:, s0:s1], recs[ti][:, s0:s1], cgt)
                        o_all = po2.tile([128, 8, 64], f32, name="o_all", tag="o")
                        for j in range(8):
                            tg, sg = acc_loc(j)
                            tl, sl = acc_loc(8 + j)
                            tmp = pc2.tile([128, 64], f32, name="tmp", tag="tmp",
                                           bufs=4)
                            nc.gpsimd.tensor_scalar_mul(
                                tmp, stg[tg][:, sg, 0:64], cf[tg][:, sg:sg + 1])
                            nc.vector.scalar_tensor_tensor(
                                o_all[:, j, :], stg[tl][:, sl, 0:64],
                                cf[tl][:, sl:sl + 1], tmp,
                                op0=mybir.AluOpType.mult,
                                op1=mybir.AluOpType.add)
                            if j == 3:
                                # first-half DMA overlaps the second half's
                                # scale ops (shortens the tail chain)
                                nc.sync.dma_start(
                                    out_d[q0:q0 + 512, h * 64:(h + 1) * 64]
                                    .rearrange("(j p) d -> p j d", p=128),
                                    o_all[:, 0:4, :])
                        nc.sync.dma_start(
                            out_d[q0 + 512:q0 + QC, h * 64:(h + 1) * 64].rearrange(
                                "(j p) d -> p j d", p=128),
                            o_all[:, 4:8, :])
                    return emit

                def v_fill(paux, st):
                    # one V seq-tile through the spare PSUM bank; interleaved
                    # into the first attention chunk so ACT's exp stream can
                    # start right after the pair-0 Q/K projection.
                    ps = paux.tile([128, 256], f32, name="vaux", tag="aux")
                    for t in range(XT_T):
                        nc.tensor.matmul(
                            ps,
                            lhsT=xt_sb[:, t, st * 128:(st + 1) * 128],
                            rhs=w_sbs["wv"][:, t, :],
                            start=(t == 0),
                            stop=(t == XT_T - 1 and not has_b))
                    if has_b:
                        nc.tensor.matmul(
                            ps, lhsT=ones_r, rhs=bv_sb, start=False, stop=True)
                    nc.vector.tensor_copy(
                        v_sb[:, st, :, 0:64],
                        ps.rearrange("p (h d) -> p h d", h=HPC))

                def qk_fill(paux, m, ci, nq, hlf):
                    # one 512-col chunk of a Q/K projection, using the single
                    # spare PSUM bank; interleaved into attention chunks so
                    # ACT never idles on a serial projection phase.
                    ps = paux.tile([128, 512], f32, name="aux", tag="aux")
                    w_sb = w_sbs["wq" if ci == 0 else "wk"]
                    col = nq * 1024 + hlf * 512
                    for t in range(XT_T):
                        nc.tensor.matmul(
                            ps, lhsT=w_sb[:, t, m * 128:(m + 1) * 128],
                            rhs=xt_sb[:, t, col:col + 512],
                            start=(t == 0), stop=(t == XT_T - 1))
                    dst = qt_sb if ci == 0 else kt_sb
                    if has_b:
                        nc.scalar.activation(
                            dst[:, m, col:col + 512], ps, AF.Identity,
                            bias=bqk_sb[:, ci, m:m + 1], scale=1.0)
                    else:
                        nc.vector.tensor_copy(dst[:, m, col:col + 512], ps)

                CHUNKS = [(h, qc) for h in range(HPC) for qc in range(2)]

                def attention3(psc, pacc, paux, sched, vfills):
                    # scores pipeline is continuous ACROSS chunk boundaries:
                    # the emitter needs only qt/kt slices, so the next
                    # chunk's first scores are issued during this chunk's
                    # last iterations and ACT never drains at a boundary.
                    sct = {}

                    def ensure_sc(gi):
                        if gi >= len(CHUNKS) * KT or gi in sct:
                            return
                        hh, qqc = CHUNKS[gi // KT]
                        tt = gi % KT
                        mm, par = hh // 2, hh % 2
                        kk = slice(64 * par, 64 * par + 64)
                        qq0 = qqc * QC
                        ps = psc.tile([128, QC], f32, name="ps", tag="sc")
                        for hlf in range(2):
                            nc.tensor.matmul(
                                ps[:, hlf * 512:(hlf + 1) * 512],
                                lhsT=kt_sb[kk, mm, tt * 128:(tt + 1) * 128],
                                rhs=qt_sb[kk, mm, qq0 + hlf * 512:
                                          qq0 + (hlf + 1) * 512],
                                start=True, stop=True)
                        sct[gi] = ps

                    for ci, (h, qc) in enumerate(CHUNKS):
                        fills = sched.get(h)
                        q0 = qc * QC
                        accs = [pacc.tile([128, sz, 65], f32, name=f"acc{ti}",
                                          tag=f"acc{ti}")
                                for ti, sz in enumerate(ACC_SIZES)]
                        if ci == 0 and vfills:
                            v_fill(paux, vfills.pop(0))
                            v_fill(paux, vfills.pop(0))
                        ensure_sc(ci * KT)
                        ensure_sc(ci * KT + 1)
                        for t in range(KT):
                            if t == 5 and state["pending"] is not None:
                                state["pending"]()
                                state["pending"] = None
                            if vfills:
                                v_fill(paux, vfills.pop(0))
                            fa = fills.get((qc, t)) if fills else None
                            if fa is not None:
                                qk_fill(paux, *fa)
                            ensure_sc(ci * KT + t + 2)
                            ps = sct.pop(ci * KT + t)
                            e = pe.tile([128, QC], bf16, name="e", tag="e")
                            nc.scalar.activation(e, ps, AF.Exp)
                            if use_em:
                                eg = pe.tile([128, QC], bf16, name="eg", tag="e")
                                nc.vector.tensor_scalar_mul(eg, e, em_sb[:, t:t + 1])
                            else:
                                eg = e
                            st0, st1 = (t == 0), (t == KT - 1)
                            # g-branch matmuls depend only on e: emit them
                            # first so PE has work while the el muls run.
                            els = []
                            for hf in range(2):
                                el = pel.tile([128, 512], bf16, name="el", tag="el")
                                meng = nc.vector if hf == 0 else nc.gpsimd
                                meng.tensor_mul(
                                    el, e[:, hf * 512:(hf + 1) * 512],
                                    msk_sb[:, t, q0 + hf * 512:q0 + (hf + 1) * 512])
                                els.append(el)
                            # start=True zeroes the whole 2KB bank, so only
                            # the bank's first group may set it.
                            for j in range(8):
                                ti, sl = acc_loc(j)
                                nc.tensor.matmul(
                                    accs[ti][:, sl, :],
                                    lhsT=eg[:, j * 128:(j + 1) * 128],
                                    rhs=v_sb[:, t, h, :],
                                    start=st0 and j in ACC_FIRST, stop=st1)
                            for j in range(8):
                                ti, sl = acc_loc(8 + j)
                                hf, j4 = divmod(j, 4)
                                nc.tensor.matmul(
                                    accs[ti][:, sl, :],
                                    lhsT=els[hf][:, j4 * 128:(j4 + 1) * 128],
                                    rhs=v_sb[:, t, h, :],
                                    start=st0 and (8 + j) in ACC_FIRST, stop=st1)
                        stg = {}
                        last_ci = ci == len(CHUNKS) - 1
                        for ti, acc in enumerate(accs):
                            s = pst.tile([128, ACC_SIZES[ti], 65], f32,
                                         name=f"stg{ti}", tag=f"stg{ti}")
                            if last_ci and ti == 1:
                                # final chunk: ACT is idle, split the copies
                                # so the tail chain shortens
                                nc.scalar.activation(s, acc, AF.Copy)
                            else:
                                nc.vector.tensor_copy(s, acc)
                            stg[ti] = s
                        state["pending"] = make_epi3(stg, h, qc)

                proj_qk3(0)
                with ExitStack() as att3:
                    psc = att3.enter_context(
                        tc.tile_pool(name="psc", bufs=2, space="PSUM"))
                    pacc = att3.enter_context(
                        tc.tile_pool(name="pacc", bufs=1, space="PSUM"))
                    paux = att3.enter_context(
                        tc.tile_pool(name="paux", bufs=1, space="PSUM"))
                    # in-attention projection fills {(qc, t): (m, ci, nq, hlf)}:
                    # h1 hosts pair-1's kt + qt-nq0 (needed before h2); h2
                    # hosts pair-1's qt-nq1 (needed only by its second chunk).
                    sched = {
                        1: {(0, 1): (1, 1, 0, 0), (0, 5): (1, 1, 0, 1),
                            (0, 9): (1, 1, 1, 0), (0, 13): (1, 1, 1, 1),
                            (1, 1): (1, 0, 0, 0), (1, 5): (1, 0, 0, 1)},
                        2: {(0, 1): (1, 0, 1, 0), (0, 5): (1, 0, 1, 1)},
                    }
                    vfills = list(range(KT))
                    attention3(psc, pacc, paux, sched, vfills)
                    assert not vfills
                    # flush the last epilogue before the PSUM pool close so
                    # its ops don't queue behind the dealloc syncs
                    if state["pending"] is not None:
                        state["pending"]()
                        state["pending"] = None
                rep3.close()
                continue

            if v2:
                # t-outer ordering: 4 concurrent PSUM chunks (8 banks) consume
                # each xt/w tile as it lands instead of waiting for all 8.
                with tc.tile_pool(name="pproj", bufs=1, space="PSUM") as pproj:
                    for nq in range(S // 1024):
                        chunks = []
                        for m in range(MPC):
                            for ci, (wn, dst) in enumerate(
                                    (("wq", qt_sb), ("wk", kt_sb))):
                                ps = pproj.tile([128, 1024], f32, tag=f"pp{m}{ci}")
                                chunks.append((m, ci, wn, dst, ps))
                        for t in range(XT_T):
                            for m, ci, wn, dst, ps in chunks:
                                w_sb = w_sbs[wn]
                                for hlf in range(2):
                                    nc.tensor.matmul(
                                        ps[:, hlf * 512:(hlf + 1) * 512],
                                        lhsT=w_sb[:, t, m * 128:(m + 1) * 128],
                                        rhs=xt_sb[:, t, nq * 1024 + hlf * 512:
                                                  nq * 1024 + (hlf + 1) * 512],
                                        start=(t == 0),
                                        stop=(t == XT_T - 1),
                                    )
                        for m, ci, wn, dst, ps in chunks:
                            nc.scalar.activation(
                                dst[:, m, nq * 1024:(nq + 1) * 1024], ps,
                                AF.Identity, bias=bqk_sb[:, ci, m:m + 1], scale=1.0,
                            )
            else:
                with tc.tile_pool(name="pproj", bufs=2, space="PSUM") as pproj:
                    for m in range(MPC):
                        for ci, (wn, dst) in enumerate((("wq", qt_sb), ("wk", kt_sb))):
                            w_sb = w_sbs[wn]
                            for nq in range(S // 1024):
                                ps = pproj.tile([128, 1024], f32, tag="pp")
                                for t in range(XT_T):
                                    for hlf in range(2):
                                        nc.tensor.matmul(
                                            ps[:, hlf * 512:(hlf + 1) * 512],
                                            lhsT=w_sb[:, t, m * 128:(m + 1) * 128],
                                            rhs=xt_sb[:, t, nq * 1024 + hlf * 512:
                                                      nq * 1024 + (hlf + 1) * 512],
                                            start=(t == 0),
                                            stop=(t == XT_T - 1),
                                        )
                                nc.scalar.activation(
                                    dst[:, m, nq * 1024:(nq + 1) * 1024], ps,
                                    AF.Identity, bias=bqk_sb[:, ci, m:m + 1], scale=1.0,
                                )
            with tc.tile_pool(name="pv", bufs=2, space="PSUM") as pv:
                for st in range(KT):
                    ps = pv.tile([128, 256], f32, tag="pv")
                    for t in range(XT_T):
                        nc.tensor.matmul(
                            ps,
                            lhsT=xt_sb[:, t, st * 128:(st + 1) * 128],
                            rhs=w_sbs["wv"][:, t, :],
                            start=(t == 0),
                            stop=(t == XT_T - 1 and not has_b),
                        )
                    if has_b:
                        nc.tensor.matmul(
                            ps, lhsT=ones_r, rhs=bv_sb, start=False, stop=True
                        )
                    nc.scalar.activation(
                        v_sb[:, st, :, 0:64],
                        ps.rearrange("p (h d) -> p h d", h=HPC),
                        AF.Copy,
                    )

            if v2:
                # ---- dual attention, v2: software-pipelined t-loop ----
                # scores in 512-wide PSUM halves (4 bufs = lookahead `la`),
                # epilogue entirely off the PE (Pool copies + broadcasts).
                att2 = ExitStack()
                psc = att2.enter_context(tc.tile_pool(name="psc", bufs=la + 1, space="PSUM"))
                pctx = att2.enter_context(tc.tile_pool(name="pctx", bufs=1, space="PSUM"))
                pe = att2.enter_context(tc.tile_pool(name="pe", bufs=10))
                pt = att2.enter_context(tc.tile_pool(name="pt", bufs=2))
                po = att2.enter_context(tc.tile_pool(name="po", bufs=2))
                pc = att2.enter_context(tc.tile_pool(name="pc", bufs=2))
                NH2 = QC // hw2
                nhalves = KT * NH2

                def make_epi(ctl_s, ctg_s, h, qs, split):
                    def emit():
                        ncol = qs.stop - qs.start
                        nsp = 2 if split else 1
                        w = ncol // nsp
                        for sp in range(nsp):
                            cs = slice(sp * w, (sp + 1) * w)
                            gsl = gt2_sb[0:1, h, qs.start + sp * w:
                                         qs.start + (sp + 1) * w]
                            rl = pc.tile([1, w], f32, name="rl", tag=f"rl{sp}")
                            rg = pc.tile([1, w], f32, name="rg", tag=f"rg{sp}")
                            nc.vector.reciprocal_approx_fast(rl, ctl_s[64:65, cs])
                            nc.vector.reciprocal_approx_fast(rg, ctg_s[64:65, cs])
                            # coef_l = g/den_l ; coef_g = rec_g - g*rec_g  (Pool)
                            cfl = pc.tile([1, w], bf16, name="cfl", tag=f"cf{sp}")
                            cfg = pc.tile([1, w], bf16, name="cfg", tag=f"cf{sp}")
                            nc.gpsimd.tensor_mul(cfl, rl, gsl)
                            cgt = pc.tile([1, w], f32, name="cgt", tag=f"cgt{sp}")
                            nc.gpsimd.tensor_mul(cgt, rg, gsl)
                            nc.gpsimd.tensor_sub(cfg, rg, cgt)
                            bcl = pt.tile([64, w], bf16, name="bcl", tag=f"bc{sp}")
                            bcg = pt.tile([64, w], bf16, name="bcg", tag=f"bc{sp}")
                            nc.gpsimd.partition_broadcast(bcl, cfl)
                            nc.gpsimd.partition_broadcast(bcg, cfg)
                            nc.vector.tensor_mul(ctl_s[0:64, cs], ctl_s[0:64, cs], bcl)
                            nc.vector.tensor_mul(ctg_s[0:64, cs], ctg_s[0:64, cs], bcg)
                            o = po.tile([64, w], f32, name="o", tag=f"o{sp}")
                            nc.vector.tensor_add(o, ctl_s[0:64, cs], ctg_s[0:64, cs])
                            nc.sync.dma_start(
                                out_d[h * 64:(h + 1) * 64,
                                      qs.start + sp * w:qs.start + (sp + 1) * w], o)
                    return emit

                pending_epi = None
                for h in range(HPC):
                    m, par = h // 2, h % 2
                    ksl = slice(64 * par, 64 * par + 64)
                    for qc in range(S // QC):
                        q0 = qc * QC
                        qs = slice(q0, q0 + QC)
                        ctg = pctx.tile([65, QC], f32, name="ctg", tag="ctxg")
                        ctl = pctx.tile([65, QC], f32, name="ctl", tag="ctxl")

                        def emit_sc(i):
                            t, hf = divmod(i, NH2)
                            ps_i = psc.tile([128, hw2], f32, name="ps", tag="sc")
                            nc.tensor.matmul(
                                ps_i,
                                lhsT=kt_sb[ksl, m, t * 128:(t + 1) * 128],
                                rhs=qt_sb[ksl, m, q0 + hf * hw2:q0 + (hf + 1) * hw2],
                                start=True, stop=True)
                            return ps_i

                        pstiles = {}
                        for j in range(min(la, nhalves)):
                            pstiles[j] = emit_sc(j)
                        for i in range(nhalves):
                            t, hf = divmod(i, NH2)
                            if i == 10 and pending_epi is not None:
                                # deferred epilogue of the previous chunk:
                                # emitted here so its DVE/Pool ops queue
                                # behind this chunk's first el muls.
                                pending_epi()
                                pending_epi = None
                            if i + la < nhalves:
                                pstiles[i + la] = emit_sc(i + la)
                            ps_i = pstiles.pop(i)
                            h5 = slice(hf * hw2, (hf + 1) * hw2)
                            e = pe.tile([128, hw2], bf16, name="e", tag="e")
                            nc.scalar.activation(e, ps_i, AF.Exp)
                            el = pe.tile([128, hw2], bf16, name="el", tag="e")
                            # alternate the mask mul between DVE and Pool
                            meng = nc.vector if i % 2 == 0 else nc.gpsimd
                            meng.tensor_mul(
                                el, e, msk_sb[:, t, q0 + hf * hw2:q0 + (hf + 1) * hw2])
                            if use_em:
                                eg = pe.tile([128, hw2], bf16, name="eg", tag="e")
                                nc.vector.tensor_scalar_mul(eg, e, em_sb[:, t:t + 1])
                            else:
                                eg = e
                            st0, st1 = (t == 0), (t == KT - 1)
                            nc.tensor.matmul(ctg[:, h5], lhsT=v_sb[:, t, h, :],
                                             rhs=eg, start=st0, stop=st1)
                            nc.tensor.matmul(ctl[:, h5], lhsT=v_sb[:, t, h, :],
                                             rhs=el, start=st0, stop=st1)
                        # copies free the ctx PSUM promptly; the rest of the
                        # epilogue is deferred into the next chunk's loop.
                        ctl_s = pt.tile([65, QC], f32, name="ctl_s", tag="cts")
                        ctg_s = pt.tile([65, QC], f32, name="ctg_s", tag="cts")
                        nc.scalar.activation(ctl_s, ctl, AF.Copy)
                        nc.vector.tensor_copy(ctg_s, ctg)
                        pending_epi = make_epi(ctl_s, ctg_s, h, qs, split=False)
                # final epilogue: split into column halves to shorten the
                # serial tail chain.
                make_epi(ctl_s, ctg_s, HPC - 1, qs, split=True)()
                pending_epi = None
                att2.close()
                continue

            # ---- dual attention ----
            att_ctx = ExitStack()
            nbuf = 4 if pairs else scbufs
            psc = att_ctx.enter_context(tc.tile_pool(name="psc", bufs=nbuf, space="PSUM"))
            pctx = att_ctx.enter_context(tc.tile_pool(name="pctx", bufs=ctxbufs, space="PSUM"))
            pe = att_ctx.enter_context(tc.tile_pool(name="pe", bufs=8 if tune else 6))
            pt = att_ctx.enter_context(tc.tile_pool(name="pt", bufs=2))
            po = att_ctx.enter_context(tc.tile_pool(name="po", bufs=2))
            pc = att_ctx.enter_context(tc.tile_pool(name="pc", bufs=8))

            if pairs:
                Q2 = 512
                for m in range(MPC):
                    hA, hB = 2 * m, 2 * m + 1
                    for qc in range(S // Q2):
                        qs = slice(qc * Q2, (qc + 1) * Q2)
                        cps = [pctx.tile([65, Q2], f32, name=f"c{j}", tag=f"c{j}")
                               for j in range(4)]  # (Ag, Al, Bg, Bl)
                        for t in range(KT):
                            st0, st1 = (t == 0), (t == KT - 1)
                            psA = psc.tile([128, Q2], f32, name="psA", tag="sc")
                            psB = psc.tile([128, Q2], f32, name="psB", tag="sc")
                            nc.tensor.matmul(
                                psA, lhsT=kt_sb[0:64, m, t * 128:(t + 1) * 128],
                                rhs=qt_sb[0:64, m, qs], start=True, stop=True)
                            nc.tensor.matmul(
                                psB, lhsT=kt_sb[64:128, m, t * 128:(t + 1) * 128],
                                rhs=qt_sb[64:128, m, qs], start=True, stop=True)
                            eA = pe.tile([128, Q2], bf16, name="eA", tag="e")
                            eB = pe.tile([128, Q2], bf16, name="eB", tag="e")
                            nc.scalar.activation(eA, psA, AF.Exp)
                            nc.scalar.activation(eB, psB, AF.Exp)
                            elA = pe.tile([128, Q2], bf16, name="elA", tag="e")
                            elB = pe.tile([128, Q2], bf16, name="elB", tag="e")
                            nc.vector.tensor_mul(elA, eA, msk_sb[:, t, qs])
                            nc.vector.tensor_mul(elB, eB, msk_sb[:, t, qs])
                            if use_em:
                                egA = pe.tile([128, Q2], bf16, name="egA", tag="e")
                                egB = pe.tile([128, Q2], bf16, name="egB", tag="e")
                                nc.vector.tensor_scalar_mul(egA, eA, em_sb[:, t:t + 1])
                                nc.vector.tensor_scalar_mul(egB, eB, em_sb[:, t:t + 1])
                            else:
                                egA, egB = eA, eB
                            for j, ee in ((0, egA), (1, elA), (2, egB), (3, elB)):
                                nc.tensor.matmul(
                                    cps[j], lhsT=v_sb[:, t, 2 * m + j // 2, :],
                                    rhs=ee, start=st0, stop=st1)
                        # epilogue (rows: l_A, g_A, l_B, g_B)
                        stage = pc.tile([65, 4, Q2], f32, name="stage",
                                        tag="stage", bufs=2)
                        for j, src in enumerate((cps[1], cps[0], cps[3], cps[2])):
                            nc.scalar.activation(stage[64:65, j, :],
                                                 src[64:65, :], AF.Copy)
                        sums4 = pc.tile([4, Q2], f32, name="sums4", tag="sums", bufs=2)
                        nc.sync.dma_start(sums4, stage[64:65, :, :])
                        rec4 = pc.tile([4, Q2], f32, name="rec4", tag="sums", bufs=2)
                        nc.vector.reciprocal_approx_fast(rec4, sums4)
                        gtt4 = pc.tile([4, Q2], f32, name="gtt4", tag="gtt", bufs=2)
                        nc.sync.dma_start(gtt4[0:2, :], gt_d[:, hA, qs])
                        nc.sync.dma_start(gtt4[2:4, :], gt_d[:, hB, qs])
                        coef4 = pc.tile([4, Q2], bf16, name="coef4",
                                        tag="coefb", bufs=2)
                        nc.vector.tensor_mul(coef4, rec4, gtt4)
                        for jj, hh in ((0, hA), (1, hB)):
                            ctg2, ctl2 = cps[2 * jj], cps[2 * jj + 1]
                            bcl = psc.tile([64, Q2], f32, name="bcl", tag="sc")
                            bcg = psc.tile([64, Q2], f32, name="bcg", tag="sc")
                            nc.tensor.matmul(bcl, lhsT=sel_sb[:, 2 * jj, :],
                                             rhs=coef4, start=True, stop=True)
                            nc.tensor.matmul(bcg, lhsT=sel_sb[:, 2 * jj + 1, :],
                                             rhs=coef4, start=True, stop=True)
                            bcl_s = pt.tile([64, Q2], f32, name="bcl_s", tag="bc")
                            bcg_s = pt.tile([64, Q2], f32, name="bcg_s", tag="bc")
                            nc.scalar.activation(bcl_s, bcl, AF.Copy)
                            nc.scalar.activation(bcg_s, bcg, AF.Copy)
                            t1 = pt.tile([64, Q2], f32, name="t1", tag="t")
                            t2 = pt.tile([64, Q2], f32, name="t2", tag="t")
                            nc.vector.tensor_mul(t1, ctl2[0:64, :], bcl_s)
                            nc.vector.tensor_mul(t2, ctg2[0:64, :], bcg_s)
                            o = po.tile([64, Q2], f32, name="o", tag="o")
                            nc.vector.tensor_add(o, t1, t2)
                            nc.sync.dma_start(out_d[hh * 64:(hh + 1) * 64, qs], o)
                att_ctx.close()
                continue

            for h in range(HPC):
                m, par = h // 2, h % 2
                ksl = slice(64 * par, 64 * par + 64)  # head's dims within the pair
                for qc in range(S // qcw):
                    qs = slice(qc * qcw, (qc + 1) * qcw)
                    ctg = pctx.tile([65, qcw], f32, name="ctg", tag="ctxg")
                    ctl = pctx.tile([65, qcw], f32, name="ctl", tag="ctxl")
                    for t in range(KT):
                        st0 = (t == 0)
                        st1 = (t == KT - 1)
                        ps = psc.tile([128, qcw], f32, name="ps", tag="sc")
                        for hlf in range(qcw // 512):
                            nc.tensor.matmul(
                                ps[:, hlf * 512:(hlf + 1) * 512],
                                lhsT=kt_sb[ksl, m, t * 128:(t + 1) * 128],
                                rhs=qt_sb[ksl, m, qc * qcw + hlf * 512:
                                          qc * qcw + (hlf + 1) * 512],
                                start=True, stop=True,
                            )
                        e = pe.tile([128, qcw], bf16, name="e", tag="e")
                        nc.scalar.activation(e, ps, AF.Exp)
                        el = e
                        if abl != "noloc":
                            el = pe.tile([128, qcw], bf16, name="el", tag="e")
                            if tune:
                                for hlf in range(qcw // 512):
                                    h5 = slice(hlf * 512, (hlf + 1) * 512)
                                    nc.vector.tensor_mul(el[:, h5], e[:, h5],
                                                         msk_sb[:, t, qc * qcw + hlf * 512:
                                                                qc * qcw + (hlf + 1) * 512])
                            else:
                                nc.vector.tensor_mul(el, e, msk_sb[:, t, qs])
                        if use_em:
                            eg = pe.tile([128, qcw], bf16, name="eg", tag="e")
                            nc.vector.tensor_scalar_mul(eg, e, em_sb[:, t:t + 1])
                        else:
                            eg = e
                        if abl == "noctx":
                            continue
                        for hlf in range(qcw // 512):
                            h5 = slice(hlf * 512, (hlf + 1) * 512)
                            nc.tensor.matmul(ctg[:, h5], lhsT=v_sb[:, t, h, :],
                                             rhs=eg[:, h5], start=st0, stop=st1)
                            if abl != "noloc":
                                nc.tensor.matmul(ctl[:, h5], lhsT=v_sb[:, t, h, :],
                                                 rhs=el[:, h5], start=st0, stop=st1)
                    if abl:
                        o = po.tile([64, qcw], f32, name="o", tag="o")
                        src_abl = el[0:64, :] if abl == "noctx" else ctg[0:64, :]
                        nc.scalar.activation(o, src_abl, AF.Copy)
                        nc.sync.dma_start(out_d[h * 64:(h + 1) * 64, qs], o)
                        continue
                    if tune:
                        # release ctx PSUM early: copy both ctx tiles to SBUF, then run
                        # the whole epilogue from SBUF while the next head accumulates.
                        ctl_s = pt.tile([65, qcw], f32, name="ctl_s", tag="cts")
                        ctg_s = pt.tile([65, qcw], f32, name="ctg_s", tag="cts")
                        nc.scalar.activation(ctl_s, ctl, AF.Copy)
                        nc.scalar.activation(ctg_s, ctg, AF.Copy)
                        sums2 = pc.tile([2, qcw], f32, name="sums2", tag="sums", bufs=2)
                        nc.sync.dma_start(sums2[0:1, :], ctl_s[64:65, :])
                        nc.sync.dma_start(sums2[1:2, :], ctg_s[64:65, :])
                        rec2 = pc.tile([2, qcw], f32, name="rec2", tag="sums", bufs=2)
                        nc.vector.reciprocal_approx_fast(rec2, sums2)
                        gtt = pc.tile([2, qcw], f32, name="gtt", tag="gtt", bufs=2)
                        nc.sync.dma_start(gtt, gt_d[:, h, qs])
                        if gbc:
                            sl1 = pc.tile([1, qcw], f32, name="sl1", tag="s1", bufs=2)
                            sg1 = pc.tile([1, qcw], f32, name="sg1", tag="s1", bufs=2)
                            nc.sync.dma_start(sl1, ctl_s[64:65, :])
                            nc.sync.dma_start(sg1, ctg_s[64:65, :])
                            rl1 = pc.tile([1, qcw], f32, name="rl1", tag="s1", bufs=2)
                            rg1 = pc.tile([1, qcw], f32, name="rg1", tag="s1", bufs=2)
                            nc.vector.reciprocal_approx_fast(rl1, sl1)
                            nc.vector.reciprocal_approx_fast(rg1, sg1)
                            gl1 = pc.tile([1, qcw], f32, name="gl1", tag="s1", bufs=2)
                            gg1 = pc.tile([1, qcw], f32, name="gg1", tag="s1", bufs=2)
                            nc.sync.dma_start(gl1, gt_d[0:1, h, qs])
                            nc.sync.dma_start(gg1, gt_d[1:2, h, qs])
                            cfl = pc.tile([1, qcw], bf16, name="cfl", tag="coefb", bufs=2)
                            cfg = pc.tile([1, qcw], bf16, name="cfg", tag="coefb", bufs=2)
                            nc.vector.tensor_mul(cfl, rl1, gl1)
                            nc.vector.tensor_mul(cfg, rg1, gg1)
                            bcl_s = pt.tile([64, qcw], bf16, name="bcl_s", tag="bcs")
                            bcg_s = pt.tile([64, qcw], bf16, name="bcg_s", tag="bcs")
                            nc.gpsimd.partition_broadcast(bcl_s, cfl)
                            nc.gpsimd.partition_broadcast(bcg_s, cfg)
                            t1 = pt.tile([64, qcw], f32, name="t1", tag="t")
                            t2 = pt.tile([64, qcw], f32, name="t2", tag="t")
                            nc.vector.tensor_mul(t1, ctl_s[0:64, :], bcl_s)
                            nc.vector.tensor_mul(t2, ctg_s[0:64, :], bcg_s)
                        else:
                            coef2 = pc.tile([2, qcw], bf16, name="coef2", tag="coefb", bufs=2)
                            nc.vector.tensor_mul(coef2, rec2, gtt)
                            bcl = psc.tile([64, qcw], f32, name="bcl", tag="sc")
                            bcg = psc.tile([64, qcw], f32, name="bcg", tag="sc")
                            for hlf in range(qcw // 512):
                                hs512 = slice(hlf * 512, (hlf + 1) * 512)
                                nc.tensor.matmul(bcl[:, hs512], lhsT=sel_sb[0:2, 0, :],
                                                 rhs=coef2[:, hs512], start=True, stop=True)
                                nc.tensor.matmul(bcg[:, hs512], lhsT=sel_sb[0:2, 1, :],
                                                 rhs=coef2[:, hs512], start=True, stop=True)
                            t1 = pt.tile([64, qcw], f32, name="t1", tag="t")
                            t2 = pt.tile([64, qcw], f32, name="t2", tag="t")
                            nc.vector.tensor_mul(t1, ctl_s[0:64, :], bcl)
                            nc.vector.tensor_mul(t2, ctg_s[0:64, :], bcg)
                        o = po.tile([64, qcw], f32, name="o", tag="o")
                        nc.vector.tensor_add(o, t1, t2)
                        nc.sync.dma_start(out_d[h * 64:(h + 1) * 64, qs], o)
                        continue
                    # epilogue: sums (psum row 64) -> [2, qcw] at base partition 0,
                    # recip * gate, broadcast via K=2 selector matmuls, combine.
                    stage = pc.tile([65, 2, qcw], f32, name="stage", tag="stage",
                                    bufs=1 if tune else 2)
                    if tune:
                        nc.vector.tensor_copy(stage[64:65, 0, :], ctl[64:65, :])
                        nc.vector.tensor_copy(stage[64:65, 1, :], ctg[64:65, :])
                    else:
                        nc.scalar.activation(stage[64:65, 0, :], ctl[64:65, :], AF.Copy)
                        nc.scalar.activation(stage[64:65, 1, :], ctg[64:65, :], AF.Copy)
                    sums2 = pc.tile([2, qcw], f32, name="sums2", tag="sums", bufs=2)
                    nc.sync.dma_start(sums2, stage[64:65, :, :])
                    rec2 = pc.tile([2, qcw], f32, name="rec2", tag="sums", bufs=2)
                    nc.vector.reciprocal_approx_fast(rec2, sums2)
                    gtt = pc.tile([2, qcw], f32, name="gtt", tag="gtt", bufs=2)
                    nc.sync.dma_start(gtt, gt_d[:, h, qs])
                    coef2 = pc.tile([2, qcw], bf16, name="coef2", tag="coefb", bufs=2)
                    nc.vector.tensor_mul(coef2, rec2, gtt)
                    bcl = psc.tile([64, qcw], f32, name="bcl", tag="sc")
                    bcg = psc.tile([64, qcw], f32, name="bcg", tag="sc")
                    for hlf in range(qcw // 512):
                        hs512 = slice(hlf * 512, (hlf + 1) * 512)
                        nc.tensor.matmul(
                            bcl[:, hs512],
                            lhsT=sel_sb[0:2, 0, :],
                            rhs=coef2[:, hs512],
                            start=True, stop=True)
                        nc.tensor.matmul(
                            bcg[:, hs512],
                            lhsT=sel_sb[0:2, 1, :],
                            rhs=coef2[:, hs512],
                            start=True, stop=True)
                    bcl_s = pt.tile([64, qcw], f32, name="bcl_s", tag="bc")
                    bcg_s = pt.tile([64, qcw], f32, name="bcg_s", tag="bc")
                    nc.scalar.activation(bcl_s, bcl, AF.Copy)
                    nc.scalar.activation(bcg_s, bcg, AF.Copy)
                    t1 = pt.tile([64, qcw], f32, name="t1", tag="t")
                    t2 = pt.tile([64, qcw], f32, name="t2", tag="t")
                    nc.vector.tensor_mul(t1, ctl[0:64, :], bcl_s)
                    nc.vector.tensor_mul(t2, ctg[0:64, :], bcg_s)
                    o = po.tile([64, qcw], f32, name="o", tag="o")
                    nc.vector.tensor_add(o, t1, t2)
                    nc.sync.dma_start(out_d[h * 64:(h + 1) * 64, qs], o)
            att_ctx.close()

    nc.compile()
    return nc


NKB = XT_T // 2  # DoubleRow K-blocks (256 rows each)
WSCALE = 64.0
ESCALE = 2.0 ** -15  # exp(q64 . k64 * 2^-15) == exp(q.k/8)


def _build4(repeat: int = 1):
    from contextlib import ExitStack

    import concourse.mybir as mybir
    from concourse import bacc, tile

    f32 = mybir.dt.float32
    bf16 = mybir.dt.bfloat16
    f8 = mybir.dt.float8e4
    AF = mybir.ActivationFunctionType
    DR = mybir.MatmulPerfMode.DoubleRow

    nc = bacc.Bacc("TRN2", target_bir_lowering=False, debug=False)

    x8_d = nc.dram_tensor("x8", [H, S], f8, kind="ExternalInput").ap()
    rx8_d = nc.dram_tensor("rx8", [H, S], f8, kind="ExternalInput").ap()
    # per-W: cols 0:256 = w8, 256:512 = rw8 (packed so DMA rows are 512B)
    w_ds = {
        nm: nc.dram_tensor(nm, [H, 512], f8, kind="ExternalInput").ap()
        for nm in ("wq", "wk", "wv")
    }
    msk_d = nc.dram_tensor("msk", [KT, 128, S], bf16, kind="ExternalInput").ap()
    gp_d = nc.dram_tensor("gp", [128, HPC, S // 128], f32, kind="ExternalInput").ap()
    out_d = nc.dram_tensor("out", [S, HPC * DH], f32, kind="ExternalOutput").ap()

    with tile.TileContext(nc) as tc, ExitStack() as ctx:
        big = ctx.enter_context(tc.tile_pool(name="big", bufs=1))

        x8_sb = big.tile([128, XT_T, S], f8, name="x8_sb")
        rx8_sb = big.tile([128, XT_T, S], f8, name="rx8_sb")
        w_sbs = {nm: big.tile([128, XT_T, 512], f8, name=f"{nm}_sb")
                 for nm in ("wq", "wk", "wv")}
        msk_sb = big.tile([128, KT, S], bf16, name="msk_sb")
        gp_sb = big.tile([128, HPC, S // 128], f32, name="gp_sb")

        # DMAs issued from different engines run on independent HW queues and
        # their transfers overlap (measured ~2x aggregate). SP carries the
        # critical projection feed (w + x8) then the q-half-2 mask; Pool
        # (idle until the first el-mul at ~10us) carries rx8 + the q-half-1
        # mask; ACT issues nothing so the exp stream starts at ~7us.
        dma = nc.sync.dma_start
        pdma = nc.gpsimd.dma_start

        def x8_slice(s0):
            dma(x8_sb[:, :, s0:s0 + 512],
                x8_d[:, s0:s0 + 512].rearrange("(t p) s -> p t s", p=128))

        def rx8_slice(s0):
            pdma(rx8_sb[:, :, s0:s0 + 512],
                 rx8_d[:, s0:s0 + 512].rearrange("(t p) s -> p t s", p=128))

        def msk_q(eng, t, qh):
            eng(msk_sb[:, t:t + 2, qh * 1024:(qh + 1) * 1024],
                msk_d[t:t + 2, :, qh * 1024:(qh + 1) * 1024]
                .rearrange("t p s -> p t s"))

        # SP: w + x8 tail (x8-s0 rides on Pool so Q0a's three inputs land in
        # parallel at ~1.6us), then the rest of the q-half-1 mask (paced just
        # ahead of chunk-0's el-mul cadence), then the q-half-2 mask (needed
        # from chunk 2, ~45us).
        dma(w_sbs["wq"], w_ds["wq"].rearrange("(t p) c -> p t c", p=128))
        dma(w_sbs["wk"], w_ds["wk"].rearrange("(t p) c -> p t c", p=128))
        x8_slice(512)
        dma(w_sbs["wv"], w_ds["wv"].rearrange("(t p) c -> p t c", p=128))
        x8_slice(1024)
        x8_slice(1536)
        for t in range(4, KT, 2):
            msk_q(dma, t, 0)
        for t in range(0, KT, 2):
            msk_q(dma, t, 1)
        # Pool: x8-s0 + rx8 + the first mask quarters (el-muls on Pool start
        # ~9us, lagging quarters are absorbed by the el tile ring)
        pdma(x8_sb[:, :, 0:512],
             x8_d[:, 0:512].rearrange("(t p) s -> p t s", p=128))
        rx8_slice(0)
        rx8_slice(512)
        msk_q(pdma, 0, 0)
        rx8_slice(1024)
        rx8_slice(1536)
        msk_q(pdma, 2, 0)
        pdma(gp_sb, gp_d)

        qt_sb = big.tile([128, MPC, S], bf16, name="qt_sb")
        kt_sb = big.tile([128, MPC, S], bf16, name="kt_sb")
        v_sb = big.tile([128, KT, HPC, 65], bf16, name="v_sb")
        nc.vector.memset(v_sb[:, :, :, 64:65], WSCALE)
        # NOTE: an fp8 DoubleRow ctx for chunk 0 ([e8|e8] x [v8|rv8] via a
        # stride-0 broadcast lhsT) simmed at -1.6us and passed a small HW
        # test, but the full kernel wedged the device
        # (NRT_EXEC_UNIT_UNRECOVERABLE) -- do not resurrect without a
        # non-broadcast e8 layout.

        def proj_term(ps, wname, wpart, rcols, term, start_first, stop_last):
            """One 4-matmul DoubleRow term of a projection chunk.

            ps: psum [128, n]; wname/wpart: which W, which 128-col slice
            (pair m for q/k); rcols: rhs x columns. Terms: x8*w8, x8*rw8,
            rx8*w8 (the dropped rx*rw residual is ~0.1%).
            """
            w_sb = w_sbs[wname]
            for kb in range(NKB):
                ks = slice(2 * kb, 2 * kb + 2)
                if term == 0:
                    lhsT = w_sb[:, ks, wpart]
                    rhs = x8_sb[:, ks, rcols]
                elif term == 1:
                    lhsT = w_sb[:, ks, 256 + wpart.start:256 + wpart.stop]
                    rhs = x8_sb[:, ks, rcols]
                else:
                    lhsT = w_sb[:, ks, wpart]
                    rhs = rx8_sb[:, ks, rcols]
                nc.tensor.matmul(
                    ps, lhsT=lhsT, rhs=rhs,
                    start=start_first and term == 0 and kb == 0,
                    stop=stop_last and term == 2 and kb == NKB - 1,
                    perf_mode=DR)

        def proj_mms(ps, wname, wpart, rcols, start_first, stop_last):
            for term in range(3):
                proj_term(ps, wname, wpart, rcols, term, start_first, stop_last)

        def v_mms(ps, st, start_first, stop_last):
            """V projection seq-tile st: lhsT = x cols, rhs = wv."""
            first = True
            for term in range(3):
                for kb in range(NKB):
                    ks = slice(2 * kb, 2 * kb + 2)
                    xs = x8_sb if term != 2 else rx8_sb
                    wc = slice(0, 256) if term != 1 else slice(256, 512)
                    nc.tensor.matmul(
                        ps,
                        lhsT=xs[:, ks, st * 128:(st + 1) * 128],
                        rhs=w_sbs["wv"][:, ks, wc],
                        start=start_first and first,
                        stop=stop_last and term == 2 and kb == NKB - 1,
                        perf_mode=DR)
                    first = False

        # PE p-state warmup: the cost model ramps the PE clock over the 3us
        # after PE first goes busy; a dummy matmul at t~0.3us (while the feed
        # is still in flight) starts that clock so the real projections run
        # at full clock from ~3.5us.
        wu = big.tile([1, 2], bf16, name="wu")
        nc.vector.memset(wu, 0.0)
        with tc.tile_pool(name="pwu", bufs=1, space="PSUM") as pwu:
            pswu = pwu.tile([1, 1], f32, tag="wu")
            nc.tensor.matmul(pswu, lhsT=wu[:, 0:1], rhs=wu[:, 1:2],
                             start=True, stop=True)

        for _rep in range(repeat):
            # ---- prefix: pair-0 Q,K first 1024 cols, in 512 halves so the
            # first scores wait only on the first ~1.5MB of feed ----
            with tc.tile_pool(name="ppre", bufs=1, space="PSUM") as ppre:
                psq = ppre.tile([128, QC], f32, tag="pq")
                psk = ppre.tile([128, QC], f32, tag="pk")
                for hlf in range(2):
                    cs = slice(hlf * 512, (hlf + 1) * 512)
                    proj_mms(psq[:, cs], "wq", slice(0, 128), cs, True, True)
                    proj_mms(psk[:, cs], "wk", slice(0, 128), cs, True, True)
                    # qt copy on DVE, kt on ACT (idle anyway): parallel chains
                    nc.vector.tensor_copy(qt_sb[:, 0, cs], psq[:, cs])
                    nc.scalar.activation(kt_sb[:, 0, cs], psk[:, cs], AF.Copy)

            rep4 = ExitStack()
            # deep e/el rings: exp(t) recycles the slot of t-bufs, so the
            # ring depth bounds how far el-muls/ctx may lag behind ACT
            # before the exp stream stalls (Pool runs epilogue bursts)
            pe = rep4.enter_context(tc.tile_pool(name="pe", bufs=8))
            pel = rep4.enter_context(tc.tile_pool(name="pel", bufs=12))
            pst = rep4.enter_context(tc.tile_pool(name="pst", bufs=2))
            pc2 = rep4.enter_context(tc.tile_pool(name="pc2", bufs=2))
            po2 = rep4.enter_context(tc.tile_pool(name="po2", bufs=2))
            state = {"pending": None}

            ACC_SIZES = (6, 6, 4)

            def acc_loc(a):
                if a < 6:
                    return 0, a
                if a < 12:
                    return 1, a - 6
                return 2, a - 12

            ACC_FIRST = {0, 6, 12}
            COEF_RUNS = ((0, 0, 6, "g", 0), (1, 0, 2, "g", 6),
                         (1, 2, 6, "l", 0), (2, 0, 4, "l", 4))

            def make_epi(stg, h, qc, last=False):
                def emit():
                    q0 = qc * QC
                    recs, cf = {}, {}
                    for ti, sz in enumerate(ACC_SIZES):
                        r = pc2.tile([128, sz], f32, name=f"rec{ti}",
                                     tag=f"rec{ti}")
                        nc.vector.reciprocal_approx_fast(r, stg[ti][:, :, 64])
                        recs[ti] = r
                        cf[ti] = pc2.tile([128, sz], f32, name=f"cf{ti}",
                                          tag=f"cf{ti}")
                    for ti, s0, s1, br, j0 in COEF_RUNS:
                        gs = gp_sb[:, h, qc * 8 + j0:qc * 8 + j0 + (s1 - s0)]
                        if br == "l":
                            nc.gpsimd.tensor_mul(
                                cf[ti][:, s0:s1], recs[ti][:, s0:s1], gs)
                        else:
                            cgt = pc2.tile([128, s1 - s0], f32, name="cgt",
                                           tag=f"cgt{ti}{s0}")
                            nc.gpsimd.tensor_mul(cgt, recs[ti][:, s0:s1], gs)
                            nc.gpsimd.tensor_sub(
                                cf[ti][:, s0:s1], recs[ti][:, s0:s1], cgt)
                    o_all = po2.tile([128, 8, 64], f32, name="o_all", tag="o")

                    def odma(eng, j0, j1):
                        eng.dma_start(
                            out_d[q0 + j0 * 128:q0 + j1 * 128,
                                  h * 64:(h + 1) * 64]
                            .rearrange("(j p) d -> p j d", p=128),
                            o_all[:, j0:j1, :])

                    for j in range(8):
                        tg, sg = acc_loc(j)
                        tl, sl = acc_loc(8 + j)
                        tmp = pc2.tile([128, 64], f32, name="tmp", tag="tmp",
                                       bufs=4)
                        nc.gpsimd.tensor_scalar_mul(
                            tmp, stg[tg][:, sg, 0:64], cf[tg][:, sg:sg + 1])
                        nc.vector.scalar_tensor_tensor(
                            o_all[:, j, :], stg[tl][:, sl, 0:64],
                            cf[tl][:, sl:sl + 1], tmp,
                            op0=mybir.AluOpType.mult,
                            op1=mybir.AluOpType.add)
                        if last:
                            # final chunk: 4 small DMAs alternating SP/ACT
                            # (ACT is idle by now) so the last transfer is
                            # short and overlaps the remaining scale ops
                            if j in (1, 5):
                                odma(nc.sync, j - 1, j + 1)
                            elif j in (3, 7):
                                odma(nc.scalar, j - 1, j + 1)
                        elif j == 3:
                            odma(nc.sync, 0, 4)
                    if not last:
                        odma(nc.sync, 4, 8)
                return emit

            def v_fill(paux, st):
                ps = paux.tile([128, 256], f32, name="vaux", tag="aux")
                v_mms(ps, st, True, True)
                nc.vector.tensor_copy(
                    v_sb[:, st, :, 0:64],
                    ps.rearrange("p (h d) -> p h d", h=HPC))

            def qk_fill_start(paux, m, ci, nq, hlf):
                # returns per-(quarter, term) continuation thunks: pieces are
                # emitted on consecutive iterations so a fill never delays an
                # iteration's scores by much, and each 256-wide piece fits
                # the shared 2-deep aux psum ring
                wn = "wq" if ci == 0 else "wk"
                wp = slice(m * 128, (m + 1) * 128)
                dst = qt_sb if ci == 0 else kt_sb
                live = {}

                def term(i):
                    def run():
                        if i == 0:
                            # allocate at execution order so the shared aux
                            # ring recycles in the order pieces run
                            c0 = nq * 1024 + hlf * 512
                            live[0] = (paux.tile([128, 512], f32, name="aux",
                                                 tag="aux"),
                                       slice(c0, c0 + 512))
                        ps, col = live[0]
                        proj_term(ps, wn, wp, col, i, True, True)
                        if i == 2:
                            nc.vector.tensor_copy(dst[:, m, col], ps)
                    return run
                return [term(0), term(1), term(2)]

            # chunk order: pair-0 fully first, then pair-1; within a pair,
            # (hA,q0),(hB,q0),(hA,q1),(hB,q1) so nq1 projections can fill in
            # the sibling head's first chunk.
            CHUNKS = [(0, 0), (1, 0), (0, 1), (1, 1),
                      (2, 0), (3, 0), (2, 1), (3, 1)]

            # in-attention projection fills {ci: {t: (m, w, nq, hlf)}}
            SCHED = {
                0: {1: (0, 1, 1, 0), 3: (0, 1, 1, 1)},      # K0 nq1
                1: {1: (0, 0, 1, 0), 5: (0, 0, 1, 1)},      # Q0 nq1
                2: {1: (1, 1, 0, 0), 5: (1, 1, 0, 1)},      # K1 nq0
                3: {1: (1, 0, 0, 0), 5: (1, 0, 0, 1)},      # Q1 nq0
                4: {1: (1, 1, 1, 0), 3: (1, 1, 1, 1)},      # K1 nq1
                5: {1: (1, 0, 1, 0), 5: (1, 0, 1, 1)},      # Q1 nq1
            }

            def attention(psc, pacc, paux):
                sct = {}

                def ensure_sc(gi):
                    if gi >= len(CHUNKS) * KT or gi in sct:
                        return
                    hh, qqc = CHUNKS[gi // KT]
                    tt = gi % KT
                    mm, par = hh // 2, hh % 2
                    kk = slice(64 * par, 64 * par + 64)
                    qq0 = qqc * QC
                    ps = psc.tile([128, QC], f32, name="ps", tag="sc")
                    for hlf in range(2):
                        nc.tensor.matmul(
                            ps[:, hlf * 512:(hlf + 1) * 512],
                            lhsT=kt_sb[kk, mm, tt * 128:(tt + 1) * 128],
                            rhs=qt_sb[kk, mm, qq0 + hlf * 512:
                                      qq0 + (hlf + 1) * 512],
                            start=True, stop=True)
                    sct[gi] = ps

                vfills = list(range(KT))
                fill_q = []
                for ci, (h, qc) in enumerate(CHUNKS):
                    fills = SCHED.get(ci)
                    q0 = qc * QC
                    accs = [pacc.tile([128, sz, 65], f32, name=f"acc{ti}",
                                      tag=f"acc{ti}")
                            for ti, sz in enumerate(ACC_SIZES)]
                    ensure_sc(ci * KT)
                    ensure_sc(ci * KT + 1)
                    if ci == 0:
                        for _ in range(4):
                            v_fill(paux, vfills.pop(0))
                    for t in range(KT):
                        if t == 5 and state["pending"] is not None:
                            state["pending"]()
                            state["pending"] = None
                        # scores t+2 BEFORE the fills: exp(t) has already
                        # freed the psc slot, and ACT can then exp t+1 AND
                        # t+2 while PE runs the fill (no ACT idle on
                        # fill-heavy iterations)
                        ensure_sc(ci * KT + t + 2)
                        if vfills:
                            v_fill(paux, vfills.pop(0))
                        fa = fills.get(t) if fills else None
                        if fa is not None:
                            fill_q.extend(qk_fill_start(paux, *fa))
                        if fill_q:
                            fill_q.pop(0)()
                        ps = sct.pop(ci * KT + t)
                        e = pe.tile([128, QC], bf16, name="e", tag="e")
                        nc.scalar.activation(e, ps, AF.Exp, scale=ESCALE)
                        els = []
                        for hf in range(2):
                            # hf0 on DVE (all-bf16 operands hit 2x_1p: 327ns),
                            # hf1 on Pool (372ns) in parallel
                            el = pel.tile([128, 512], bf16, name="el", tag="el")
                            meng = nc.vector if hf == 0 else nc.gpsimd
                            meng.tensor_mul(
                                el, e[:, hf * 512:(hf + 1) * 512],
                                msk_sb[:, t, q0 + hf * 512:q0 + (hf + 1) * 512])
                            els.append(el)
                        st0, st1 = (t == 0), (t == KT - 1)
                        for j in range(8):
                            ti, sl = acc_loc(j)
                            nc.tensor.matmul(
                                accs[ti][:, sl, :],
                                lhsT=e[:, j * 128:(j + 1) * 128],
                                rhs=v_sb[:, t, h, :],
                                start=st0 and j in ACC_FIRST, stop=st1)
                        for j in range(8):
                            ti, sl = acc_loc(8 + j)
                            hf, j4 = divmod(j, 4)
                            nc.tensor.matmul(
                                accs[ti][:, sl, :],
                                lhsT=els[hf][:, j4 * 128:(j4 + 1) * 128],
                                rhs=v_sb[:, t, h, :],
                                start=st0 and (8 + j) in ACC_FIRST, stop=st1)
                    last_ci = ci == len(CHUNKS) - 1
                    stg = {}
                    for ti, acc in enumerate(accs):
                        s = pst.tile([128, ACC_SIZES[ti], 65], f32,
                                     name=f"stg{ti}", tag=f"stg{ti}")
                        if last_ci and ti >= 1:
                            # final chunk: stg1/stg2 on ACT so DVE's serial
                            # tail chain is just stg0 + recs + stt's
                            # (GPSIMD cannot read PSUM)
                            nc.scalar.activation(s, acc, AF.Copy)
                        else:
                            nc.vector.tensor_copy(s, acc)
                        stg[ti] = s
                    state["pending"] = make_epi(stg, h, qc, last=last_ci)
                assert not vfills

            with ExitStack() as att:
                psc = att.enter_context(
                    tc.tile_pool(name="psc", bufs=2, space="PSUM"))
                pacc = att.enter_context(
                    tc.tile_pool(name="pacc", bufs=1, space="PSUM"))
                paux = att.enter_context(
                    tc.tile_pool(name="paux", bufs=1, space="PSUM"))
                attention(psc, pacc, paux)
                if state["pending"] is not None:
                    state["pending"]()
                    state["pending"] = None
            rep4.close()

    nc.compile()
    return nc


def _prep_core4(c, hs, lm, go, Wq, Wk, Wv):
    bf = ml_dtypes.bfloat16
    f8 = ml_dtypes.float8_e4m3fn
    b, hg = c // 4, c % 4
    h0 = hg * HPC
    sl = slice(h0 * DH, (h0 + HPC) * DH)
    xs = np.ascontiguousarray(hs[b].T).astype(np.float32)
    x8 = xs.astype(f8)
    rx8 = (xs - x8.astype(np.float32)).astype(f8)

    def wsplit(W):
        ws = np.ascontiguousarray((W[sl, :] * WSCALE).T).astype(np.float32)
        w8 = ws.astype(f8)
        rw8 = (ws - w8.astype(np.float32)).astype(f8)
        return np.concatenate([w8, rw8], axis=1)

    return {
        "x8": x8,
        "rx8": rx8,
        "wq": wsplit(Wq),
        "wk": wsplit(Wk),
        "wv": wsplit(Wv),
        "msk": np.ascontiguousarray(
            lm[b, 0].astype(np.float32).T).reshape(KT, 128, S).astype(bf),
        "gp": np.stack(
            [go[b, h0 + j, :, 0].reshape(S // 128, 128).T for j in range(HPC)],
            axis=1).astype(np.float32),
    }


def _get(use_em: bool, has_b: bool):
    key = (use_em, has_b)
    if key not in _BUILT:
        if not use_em and not has_b:
            _BUILT[key] = _build4()
        else:
            _BUILT[key] = _build(use_em, has_b=has_b)
    return _BUILT[key]


def _prep_core(c, hs, am, lm, go, Wq, bq, Wk, bk, Wv, bv, use_em):
    bf = ml_dtypes.bfloat16
    b, hg = c // 4, c % 4
    h0 = hg * HPC
    sl = slice(h0 * DH, (h0 + HPC) * DH)
    m = {
        "xt": np.ascontiguousarray(hs[b].T).astype(bf),
        "wq": np.ascontiguousarray((Wq[sl, :] / 8.0).T).astype(bf),
        "wk": np.ascontiguousarray(Wk[sl, :].T).astype(bf),
        "wv": np.ascontiguousarray(Wv[sl, :].T).astype(bf),
        "bqk": np.stack([bq[sl] / 8.0, bk[sl]]).astype(np.float32),
        "bv": bv[sl].reshape(1, 256).astype(bf),
        "msk": np.ascontiguousarray(
            lm[b, 0].astype(np.float32).T).reshape(KT, 128, S).astype(bf),
        "msk8": np.ascontiguousarray(
            lm[b, 0].astype(np.float32).T).reshape(KT, 128, S).astype(
                ml_dtypes.float8_e4m3fn),
        "gt": np.stack([
            np.stack([go[b, h0 + j, :, 0] for j in range(HPC)]),
            np.stack([1.0 - go[b, h0 + j, :, 0] for j in range(HPC)]),
        ]).astype(np.float32),
        "gt2": np.stack(
            [go[b, h0 + j, :, 0] for j in range(HPC)]
        ).reshape(1, HPC, S).astype(bf),
        "gp": np.stack(
            [go[b, h0 + j, :, 0].reshape(S // 128, 128).T for j in range(HPC)],
            axis=1).astype(np.float32),
        "sel": np.broadcast_to(
            np.eye(4, dtype=np.float32)[:, :, None], (4, 4, 64)).astype(bf),
    }
    if use_em:
        m["em"] = np.exp(am[b, 0, 0]).astype(np.float32).reshape(KT, 128)
    return m


def make_in_maps(inputs):
    hs = np.asarray(inputs["hidden_states"], np.float32)
    am = np.asarray(inputs["attention_mask"], np.float32)
    lm = np.asarray(inputs["local_attention_mask"])
    go = np.asarray(inputs["gate_outputs"], np.float32)
    Wq = np.asarray(inputs["Wq"], np.float32)
    bq = np.asarray(inputs["bq"], np.float32)
    Wk = np.asarray(inputs["Wk"], np.float32)
    bk = np.asarray(inputs["bk"], np.float32)
    Wv = np.asarray(inputs["Wv"], np.float32)
    bv = np.asarray(inputs["bv"], np.float32)
    use_em = bool(np.any(am != 0.0))
    has_b = bool(np.any(bq != 0.0) or np.any(bk != 0.0) or np.any(bv != 0.0))
    if not use_em and not has_b:
        maps = [_prep_core4(c, hs, lm, go, Wq, Wk, Wv) for c in range(NCORES)]
    else:
        maps = [
            _prep_core(c, hs, am, lm, go, Wq, bq, Wk, bk, Wv, bv, use_em)
            for c in range(NCORES)
        ]
    return maps, (use_em, has_b)


def assemble(results):
    out = np.empty((B, S, H), np.float32)
    for c in range(NCORES):
        b, hg = c // 4, c % 4
        sl = slice(hg * HPC * DH, (hg + 1) * HPC * DH)
        r = np.asarray(results[c]["out"])
        out[b, :, sl] = r if r.shape[0] == S else r.T
    return out


def kernel(**inputs):
    from concourse import bass_utils

    maps, (use_em, has_b) = make_in_maps(inputs)
    nc = _get(use_em, has_b)
    res = bass_utils.run_bass_kernel_spmd(nc, maps, core_ids=list(range(NCORES)))
    return assemble(res.results)

